# revision 5
# baseline (speedup 1.0000x reference)
"""CRF NLL loss on 8 Trainium2 NeuronCores (Bass/Tile).

Algorithm: the forward (logsumexp) recursion is run in scaled
exponential space as a chain of block-diagonal PE matmuls with a
per-step DVE merge against an exp'd emission stream. A 10th "sink"
lane per row absorbs each row's terminal mass exactly at its length
(gated by a host-built 0/1 stream), which makes variable-length rows
branch-free on device. The emission (real-path) score is a fused
multiply-reduce of bert_encode against a host-encoded one-hot of tags.
Tiny 11x11 transition-table terms (first/mid/last) are summed on host.

Data parallel: batch is split 512 rows/core across 8 cores; the scalar
numerator/denominator partials are reduced on host.
"""

import numpy as np

NT = 9
B, S = 4096, 512
NC = 8
LANES = 10            # 9 tags + sink lane
G = 12                # row groups packed on partitions
P = LANES * G         # 120 partitions used
CH = 2                # chains per core
RPC = 256             # rows per chain
F = 22                # columns per chain (ceil(256/12))
NSLICE = S + 1        # emission slices (init + 512 steps)

_CACHE = {}


def _build_module():
    from contextlib import ExitStack

    import concourse.bacc as bacc
    import concourse.mybir as mybir
    import concourse.tile as tile

    dt = mybir.dt
    AF = mybir.ActivationFunctionType
    OP = mybir.AluOpType

    nc = bacc.Bacc(None, target_bir_lowering=False, debug=False)
    em = nc.declare_dram_parameter("em", [CH, P, NSLICE * F], dt.bfloat16, isOutput=False)
    wbd = nc.declare_dram_parameter("wbd", [P, P], dt.bfloat16, isOutput=False)
    won = nc.declare_dram_parameter("won", [P, G], dt.bfloat16, isOutput=False)
    wbc = nc.declare_dram_parameter("wbc", [G, P], dt.float32, isOutput=False)
    bebm = nc.declare_dram_parameter("bebm", [4, 128, S * NT], dt.bfloat16, isOutput=False)
    oh = nc.declare_dram_parameter("oh", [4, 128, S * NT], dt.bfloat16, isOutput=False)
    orow = nc.declare_dram_parameter("orow", [CH, G, F], dt.float32, isOutput=True)
    oreal = nc.declare_dram_parameter("oreal", [4, 128, 1], dt.float32, isOutput=True)

    with tile.TileContext(nc) as tc, ExitStack() as ctx:
        const = ctx.enter_context(tc.tile_pool(name="const", bufs=1))
        wbd_t = const.tile([P, P], dt.bfloat16, tag="wbd", name="wbdt")
        nc.sync.dma_start(wbd_t[:], wbd[:])
        won_t = const.tile([P, G], dt.bfloat16, tag="won", name="wont")
        nc.sync.dma_start(won_t[:], won[:])
        wbc_t = const.tile([G, P], dt.float32, tag="wbc", name="wbct")
        nc.sync.dma_start(wbc_t[:], wbc[:])

        eep = ctx.enter_context(tc.tile_pool(name="ee", bufs=1))
        pp = ctx.enter_context(tc.tile_pool(name="pst", bufs=1))
        cp = ctx.enter_context(tc.tile_pool(name="cacc", bufs=1))
        emseg = ctx.enter_context(tc.tile_pool(name="emseg", bufs=3))
        qp = ctx.enter_context(tc.tile_pool(name="q", bufs=2, space="PSUM"))
        rp = ctx.enter_context(tc.tile_pool(name="ren", bufs=2))
        rps = ctx.enter_context(tc.tile_pool(name="renp", bufs=1, space="PSUM"))

        ee_t = [eep.tile([P, NSLICE * F], dt.bfloat16, tag=f"ee{c}", name=f"ee{c}") for c in range(CH)]
        p_t = [pp.tile([P, F], dt.bfloat16, tag=f"p{c}", name=f"p{c}") for c in range(CH)]
        c_t = [cp.tile([G, F], dt.float32, tag=f"c{c}", name=f"c{c}") for c in range(CH)]

        # stream emissions in, exponentiate on ACT (9 segments of 57 slices)
        SEG = 57
        for c in range(CH):
            for sgi in range(9):
                lo = sgi * SEG * F
                n = SEG * F
                seg = emseg.tile([P, SEG * F], dt.bfloat16, tag="seg", name="seg")
                nc.sync.dma_start(seg[:], em[c, :, lo:lo + n])
                nc.scalar.activation(ee_t[c][:, lo:lo + n], seg[:], AF.Exp)

        # init state from slice 0
        for c in range(CH):
            nc.vector.tensor_copy(p_t[c][:], ee_t[c][:, 0:F])

        # the scan: p <- (Wbd.T @ p) * ee[:, t]
        for t in range(1, S + 1):
            for c in range(CH):
                q = qp.tile([P, F], dt.float32, tag=f"q{c}", name=f"q{c}")
                nc.tensor.matmul(q[:], wbd_t[:], p_t[c][:], start=True, stop=True)
                nc.vector.tensor_tensor(
                    p_t[c][:], q[:], ee_t[c][:, t * F:(t + 1) * F], OP.mult
                )
            if t == S // 2:
                # one mid-scan renorm for fp range safety
                for c in range(CH):
                    sm = rps.tile([G, F], dt.float32, tag=f"sm{c}", name=f"sm{c}")
                    nc.tensor.matmul(sm[:], won_t[:], p_t[c][:], start=True, stop=True)
                    nc.scalar.activation(c_t[c][:], sm[:], AF.Ln)
                    r = rp.tile([G, F], dt.float32, tag=f"r{c}", name=f"r{c}")
                    nc.vector.reciprocal(r[:], sm[:])
                    rb = rps.tile([P, F], dt.float32, tag=f"rb{c}", name=f"rb{c}")
                    nc.tensor.matmul(rb[:], wbc_t[:], r[:], start=True, stop=True)
                    nc.vector.tensor_tensor(p_t[c][:], p_t[c][:], rb[:], OP.mult)

        # epilogue: orow = Ln(sink lane) + c
        for c in range(CH):
            lnA = rp.tile([G, F], dt.float32, tag=f"ln{c}", name=f"ln{c}")
            nc.scalar.activation(lnA[:], p_t[c][9 * G:10 * G, :], AF.Ln)
            nc.vector.tensor_tensor(lnA[:], lnA[:], c_t[c][:], OP.add)
            nc.sync.dma_start(orow[c], lnA[:])

        # real-path emission score: sum_t be[b,t,tags[b,t]]*mask
        bp = ctx.enter_context(tc.tile_pool(name="be", bufs=2))
        ohp = ctx.enter_context(tc.tile_pool(name="ohm", bufs=2))
        scrp = ctx.enter_context(tc.tile_pool(name="scr", bufs=2))
        accp = ctx.enter_context(tc.tile_pool(name="acc", bufs=2))
        for i in range(4):
            be_t = bp.tile([128, S * NT], dt.bfloat16, tag="be", name="bet")
            oh_t = ohp.tile([128, S * NT], dt.bfloat16, tag="oh", name="oht")
            nc.sync.dma_start(be_t[:], bebm[i])
            nc.sync.dma_start(oh_t[:], oh[i])
            scr = scrp.tile([128, S * NT], dt.bfloat16, tag="scr", name="scrt")
            acc = accp.tile([128, 1], dt.float32, tag="acc", name="acct")
            nc.vector.tensor_tensor_reduce(
                out=scr[:], in0=be_t[:], in1=oh_t[:], scale=1.0, scalar=0.0,
                op0=OP.mult, op1=OP.add, accum_out=acc[:],
            )
            nc.sync.dma_start(oreal[i], acc[:])
    return nc


def _run_device(inmaps):
    from concourse import bass_utils

    nc = _CACHE.get("nc")
    if nc is None:
        nc = _build_module()
        _CACHE["nc"] = nc
    r = bass_utils.run_bass_kernel_spmd(nc, inmaps, core_ids=list(range(NC)))
    _CACHE["last"] = r
    return r.results


def _kernel_bass(be, om, tg, T):
    import ml_dtypes

    bf16 = ml_dtypes.bfloat16
    lens = om.sum(1).astype(np.int64)
    log9 = float(np.log(NT))

    # device weights (block-diagonal, lane-major: partition = lane*G + g)
    W = np.zeros((LANES, LANES), np.float32)
    W[:NT, :NT] = np.exp(T[:NT, :NT]) / NT
    W[:NT, 9] = np.exp(T[:NT, NT + 1]) / NT
    W[9, 9] = 1.0
    wbd = np.zeros((P, P), np.float32)
    for g in range(G):
        idx = np.arange(LANES) * G + g
        wbd[np.ix_(idx, idx)] = W
    won = np.zeros((P, G), np.float32)
    won[np.arange(P), np.arange(P) % G] = 1.0
    wbc = np.zeros((G, P), np.float32)
    wbc[np.arange(P) % G, np.arange(P)] = 1.0

    # emission stream [B, 513, 10] in log space (exp'd on device)
    emfull = np.full((B, NSLICE, LANES), -10000.0, np.float32)
    emfull[:, 0, :NT] = T[NT, :NT][None, :] + be[:, 0, :]
    sidx = np.arange(1, S)
    live = sidx[None, :] < lens[:, None]          # p-lanes live: s <= L-1
    emfull[:, 1:S, :NT] = np.where(live[:, :, None], be[:, 1:, :], -10000.0)
    sall = np.arange(1, NSLICE)
    emfull[:, 1:, 9] = np.where(sall[None, :] >= lens[:, None], 0.0, -10000.0)

    # host transition-table terms (11x11 table lookups only)
    first = T[NT, tg[:, 0]]
    last = T[tg[np.arange(B), lens - 1], NT + 1]
    mid = (T[:NT, :NT][tg[:, :-1], tg[:, 1:]] * om[:, 1:]).sum(1)
    hostr = (first + last + mid).astype(np.float64)

    # one-hot of tags (mask folded in) for the device emission reduce
    ohfull = (tg[..., None] == np.arange(NT)[None, None, :]).astype(np.float32)
    ohfull *= om[..., None]

    # slot map: chain c, rank rr = col*G+g -> row c*RPC+rr
    rowidx = np.zeros((CH, G, F), np.int64)
    padm = np.zeros((CH, G, F), bool)
    for c in range(CH):
        for col in range(F):
            for g in range(G):
                rr = col * G + g
                if rr < RPC:
                    rowidx[c, g, col] = c * RPC + rr
                else:
                    padm[c, g, col] = True
    padrow = np.concatenate(
        [np.zeros((NSLICE, NT), np.float32),
         np.full((NSLICE, 1), -10000.0, np.float32)], axis=1)

    wbd16, won16 = wbd.astype(bf16), won.astype(bf16)
    inmaps = []
    for k in range(NC):
        sl = slice(k * 512, (k + 1) * 512)
        emc = emfull[sl]
        sel = emc[rowidx.reshape(-1)].reshape(CH, G, F, NSLICE, LANES).copy()
        sel[padm] = padrow
        stream = sel.transpose(0, 4, 1, 3, 2).reshape(CH, P, NSLICE * F)
        inmaps.append({
            "em": np.ascontiguousarray(stream).astype(bf16),
            "wbd": wbd16, "won": won16, "wbc": wbc,
            "bebm": be[sl].reshape(4, 128, S * NT).astype(bf16),
            "oh": ohfull[sl].reshape(4, 128, S * NT).astype(bf16),
        })

    results = _run_device(inmaps)

    num = 0.0
    den = float(lens.sum())
    # inverse slot map
    rr_all = np.arange(RPC)
    col_a, g_a = rr_all // G, rr_all % G
    for k in range(NC):
        sl = slice(k * 512, (k + 1) * 512)
        orow = np.asarray(results[k]["orow"], np.float64)   # [CH, G, F]
        oreal = np.asarray(results[k]["oreal"], np.float64).reshape(512)
        tot = np.zeros(512, np.float64)
        for c in range(CH):
            tot[c * RPC + rr_all] = orow[c, g_a, col_a]
        lc = lens[sl].astype(np.float64)
        num += float((tot + lc * log9 - oreal - hostr[sl]).sum())
    return np.float32(num / den)


def _crf_parts_np(bert_encode, transitions, output_mask, tags):
    """Numpy fallback: returns full loss."""
    ntag = NT
    start, end = ntag, ntag + 1
    maskf = output_mask.astype(np.float32)
    lengths = output_mask.sum(-1).astype(np.int64)
    b = bert_encode.shape[0]
    ar = np.arange(b)
    emit = np.take_along_axis(
        bert_encode, tags[..., None].astype(np.int64), axis=-1)[..., 0]
    emit_score = (emit * maskf).sum(-1)
    first_trans = transitions[start, tags[:, 0]]
    mid = transitions[tags[:, :-1], tags[:, 1:]]
    mid_score = (mid * maskf[:, 1:]).sum(-1)
    last_tag = tags[ar, lengths - 1]
    last_trans = transitions[last_tag, end]
    real = emit_score + first_trans + mid_score + last_trans

    trans_tt = transitions[:ntag, :ntag]
    alpha = transitions[start, :ntag][None, :] + bert_encode[:, 0, :]
    for t in range(1, bert_encode.shape[1]):
        em = bert_encode[:, t, :]
        x = alpha[:, :, None] + trans_tt[None, :, :] + em[:, None, :]
        m = x.max(axis=1)
        new = m + np.log(np.exp(x - m[:, None, :]).sum(axis=1))
        upd = output_mask[:, t] > 0
        alpha = np.where(upd[:, None], new, alpha)
    x = alpha + transitions[:ntag, end][None, :]
    m = x.max(axis=-1)
    total = m + np.log(np.exp(x - m[:, None]).sum(-1))
    return np.float32((total - real).sum() / maskf.sum())


def kernel(bert_encode, output_mask, tags, transitions):
    be = np.asarray(bert_encode, dtype=np.float32)
    om = np.asarray(output_mask, dtype=np.int32)
    tg = np.asarray(tags).astype(np.int64)
    T = np.asarray(transitions, dtype=np.float32)
    try:
        return _kernel_bass(be, om, tg.astype(np.int32), T)
    except Exception:
        import traceback
        traceback.print_exc()
        return _crf_parts_np(be, T, om, tg)


# revision 7
# speedup vs baseline: 1.8571x; 1.8571x over previous
"""CRF NLL loss on 8 Trainium2 NeuronCores (Bass/Tile).

Algorithm: the forward (logsumexp) recursion is run in scaled
exponential space as a chain of block-diagonal PE matmuls with a
per-step DVE merge against an exp'd emission stream. A 10th "sink"
lane per row absorbs each row's terminal mass exactly at its length
(gated by a host-built 0/1 stream), which makes variable-length rows
branch-free on device. The emission (real-path) score is a fused
multiply-reduce of bert_encode against a host-encoded one-hot of tags.
Tiny 11x11 transition-table terms (first/mid/last) are summed on host.

Data parallel: batch is split 512 rows/core across 8 cores; the scalar
numerator/denominator partials are reduced on host.
"""

import numpy as np

NT = 9
B, S = 4096, 512
NC = 8
LANES = 10            # 9 tags + sink lane
G = 12                # row groups packed on partitions
P = LANES * G         # 120 partitions used
CH = 2                # chains per core
RPC = 256             # rows per chain
F = 22                # columns per chain (ceil(256/12))
NSLICE = S + 1        # emission slices (init + 512 steps)

_CACHE = {}


def _build_module():
    from contextlib import ExitStack

    import concourse.bacc as bacc
    import concourse.mybir as mybir
    import concourse.tile as tile

    dt = mybir.dt
    AF = mybir.ActivationFunctionType
    OP = mybir.AluOpType

    nc = bacc.Bacc(None, target_bir_lowering=False, debug=False)
    em = nc.declare_dram_parameter("em", [CH, P, NSLICE * F], dt.bfloat16, isOutput=False)
    wbd = nc.declare_dram_parameter("wbd", [P, P], dt.bfloat16, isOutput=False)
    won = nc.declare_dram_parameter("won", [P, G], dt.bfloat16, isOutput=False)
    wbc = nc.declare_dram_parameter("wbc", [G, P], dt.float32, isOutput=False)
    bebm = nc.declare_dram_parameter("bebm", [4, 128, S * NT], dt.bfloat16, isOutput=False)
    oh = nc.declare_dram_parameter("oh", [4, 128, S * NT], dt.bfloat16, isOutput=False)
    orow = nc.declare_dram_parameter("orow", [CH, G, F], dt.float32, isOutput=True)
    oreal = nc.declare_dram_parameter("oreal", [4, 128, 1], dt.float32, isOutput=True)

    with tile.TileContext(nc) as tc, ExitStack() as ctx:
        const = ctx.enter_context(tc.tile_pool(name="const", bufs=1))
        wbd_t = const.tile([P, P], dt.bfloat16, tag="wbd", name="wbdt")
        nc.sync.dma_start(wbd_t[:], wbd[:])
        won_t = const.tile([P, G], dt.bfloat16, tag="won", name="wont")
        nc.sync.dma_start(won_t[:], won[:])
        wbc_t = const.tile([G, P], dt.float32, tag="wbc", name="wbct")
        nc.sync.dma_start(wbc_t[:], wbc[:])

        eep = ctx.enter_context(tc.tile_pool(name="ee", bufs=1))
        pp = ctx.enter_context(tc.tile_pool(name="pst", bufs=1))
        cp = ctx.enter_context(tc.tile_pool(name="cacc", bufs=1))
        emseg = ctx.enter_context(tc.tile_pool(name="emseg", bufs=3))
        qp = ctx.enter_context(tc.tile_pool(name="q", bufs=2, space="PSUM"))
        rp = ctx.enter_context(tc.tile_pool(name="ren", bufs=2))
        rps = ctx.enter_context(tc.tile_pool(name="renp", bufs=1, space="PSUM"))

        ee_t = [eep.tile([P, NSLICE * F], dt.bfloat16, tag=f"ee{c}", name=f"ee{c}") for c in range(CH)]
        p_t = [pp.tile([P, F], dt.bfloat16, tag=f"p{c}", name=f"p{c}") for c in range(CH)]
        c_t = [cp.tile([G, F], dt.float32, tag=f"c{c}", name=f"c{c}") for c in range(CH)]

        # stream emissions in, exponentiate on ACT (9 segments of 57 slices)
        SEG = 57
        for c in range(CH):
            for sgi in range(9):
                lo = sgi * SEG * F
                n = SEG * F
                seg = emseg.tile([P, SEG * F], dt.bfloat16, tag="seg", name="seg")
                nc.sync.dma_start(seg[:], em[c, :, lo:lo + n])
                nc.scalar.activation(ee_t[c][:, lo:lo + n], seg[:], AF.Exp)

        # init state from slice 0
        for c in range(CH):
            nc.vector.tensor_copy(p_t[c][:], ee_t[c][:, 0:F])

        # the scan: p <- (Wbd.T @ p) * ee[:, t]
        for t in range(1, S + 1):
            for c in range(CH):
                q = qp.tile([P, F], dt.float32, tag=f"q{c}", name=f"q{c}")
                nc.tensor.matmul(q[:], wbd_t[:], p_t[c][:], start=True, stop=True)
                nc.vector.tensor_tensor(
                    p_t[c][:], q[:], ee_t[c][:, t * F:(t + 1) * F], OP.mult
                )
            if t == S // 2:
                # one mid-scan renorm for fp range safety
                for c in range(CH):
                    sm = rps.tile([G, F], dt.float32, tag=f"sm{c}", name=f"sm{c}")
                    nc.tensor.matmul(sm[:], won_t[:], p_t[c][:], start=True, stop=True)
                    nc.scalar.activation(c_t[c][:], sm[:], AF.Ln)
                    r = rp.tile([G, F], dt.float32, tag=f"r{c}", name=f"r{c}")
                    nc.vector.reciprocal(r[:], sm[:])
                    rb = rps.tile([P, F], dt.float32, tag=f"rb{c}", name=f"rb{c}")
                    nc.tensor.matmul(rb[:], wbc_t[:], r[:], start=True, stop=True)
                    nc.vector.tensor_tensor(p_t[c][:], p_t[c][:], rb[:], OP.mult)

        # epilogue: orow = Ln(sink lane) + c
        for c in range(CH):
            lnA = rp.tile([G, F], dt.float32, tag=f"ln{c}", name=f"ln{c}")
            nc.scalar.activation(lnA[:], p_t[c][9 * G:10 * G, :], AF.Ln)
            nc.vector.tensor_tensor(lnA[:], lnA[:], c_t[c][:], OP.add)
            nc.sync.dma_start(orow[c], lnA[:])

        # real-path emission score: sum_t be[b,t,tags[b,t]]*mask
        bp = ctx.enter_context(tc.tile_pool(name="be", bufs=2))
        ohp = ctx.enter_context(tc.tile_pool(name="ohm", bufs=2))
        scrp = ctx.enter_context(tc.tile_pool(name="scr", bufs=2))
        accp = ctx.enter_context(tc.tile_pool(name="acc", bufs=2))
        for i in range(4):
            be_t = bp.tile([128, S * NT], dt.bfloat16, tag="be", name="bet")
            oh_t = ohp.tile([128, S * NT], dt.bfloat16, tag="oh", name="oht")
            nc.sync.dma_start(be_t[:], bebm[i])
            nc.sync.dma_start(oh_t[:], oh[i])
            scr = scrp.tile([128, S * NT], dt.bfloat16, tag="scr", name="scrt")
            acc = accp.tile([128, 1], dt.float32, tag="acc", name="acct")
            nc.vector.tensor_tensor_reduce(
                out=scr[:], in0=be_t[:], in1=oh_t[:], scale=1.0, scalar=0.0,
                op0=OP.mult, op1=OP.add, accum_out=acc[:],
            )
            nc.sync.dma_start(oreal[i], acc[:])
    nc.finalize()
    return nc


def _run_device(inmaps):
    from concourse import bass_utils

    nc = _CACHE.get("nc")
    if nc is None:
        nc = _build_module()
        _CACHE["nc"] = nc
    r = bass_utils.run_bass_kernel_spmd(nc, inmaps, core_ids=list(range(NC)))
    _CACHE["last"] = r
    return r.results


def _kernel_bass(be, om, tg, T):
    import ml_dtypes

    bf16 = ml_dtypes.bfloat16
    lens = om.sum(1).astype(np.int64)
    # per-step weight scale: e^-kappa; kappa = log NT + 0.5 cancels both the
    # NT-way mixing gain and E[e^N(0,1)] = sqrt(e) emission growth
    kappa = float(np.log(NT) + 0.5)

    # device weights (block-diagonal, lane-major: partition = lane*G + g)
    W = np.zeros((LANES, LANES), np.float32)
    sc = float(np.exp(-kappa))
    W[:NT, :NT] = np.exp(T[:NT, :NT]) * sc
    W[:NT, 9] = np.exp(T[:NT, NT + 1]) * sc
    W[9, 9] = 1.0
    wbd = np.zeros((P, P), np.float32)
    for g in range(G):
        idx = np.arange(LANES) * G + g
        wbd[np.ix_(idx, idx)] = W
    won = np.zeros((P, G), np.float32)
    won[np.arange(P), np.arange(P) % G] = 1.0
    wbc = np.zeros((G, P), np.float32)
    wbc[np.arange(P) % G, np.arange(P)] = 1.0

    # emission stream [B, 513, 10] in log space (exp'd on device)
    emfull = np.full((B, NSLICE, LANES), -10000.0, np.float32)
    emfull[:, 0, :NT] = T[NT, :NT][None, :] + be[:, 0, :]
    sidx = np.arange(1, S)
    live = sidx[None, :] < lens[:, None]          # p-lanes live: s <= L-1
    emfull[:, 1:S, :NT] = np.where(live[:, :, None], be[:, 1:, :], -10000.0)
    sall = np.arange(1, NSLICE)
    emfull[:, 1:, 9] = np.where(sall[None, :] >= lens[:, None], 0.0, -10000.0)

    # host transition-table terms (11x11 table lookups only)
    first = T[NT, tg[:, 0]]
    last = T[tg[np.arange(B), lens - 1], NT + 1]
    mid = (T[:NT, :NT][tg[:, :-1], tg[:, 1:]] * om[:, 1:]).sum(1)
    hostr = (first + last + mid).astype(np.float64)

    # one-hot of tags (mask folded in) for the device emission reduce
    ohfull = (tg[..., None] == np.arange(NT)[None, None, :]).astype(np.float32)
    ohfull *= om[..., None]

    # slot map: chain c, rank rr = col*G+g -> row c*RPC+rr
    rowidx = np.zeros((CH, G, F), np.int64)
    padm = np.zeros((CH, G, F), bool)
    for c in range(CH):
        for col in range(F):
            for g in range(G):
                rr = col * G + g
                if rr < RPC:
                    rowidx[c, g, col] = c * RPC + rr
                else:
                    padm[c, g, col] = True
    padrow = np.concatenate(
        [np.zeros((NSLICE, NT), np.float32),
         np.full((NSLICE, 1), -10000.0, np.float32)], axis=1)

    wbd16, won16 = wbd.astype(bf16), won.astype(bf16)
    inmaps = []
    for k in range(NC):
        sl = slice(k * 512, (k + 1) * 512)
        emc = emfull[sl]
        sel = emc[rowidx.reshape(-1)].reshape(CH, G, F, NSLICE, LANES).copy()
        sel[padm] = padrow
        stream = sel.transpose(0, 4, 1, 3, 2).reshape(CH, P, NSLICE * F)
        inmaps.append({
            "em": np.ascontiguousarray(stream).astype(bf16),
            "wbd": wbd16, "won": won16, "wbc": wbc,
            "bebm": be[sl].reshape(4, 128, S * NT).astype(bf16),
            "oh": ohfull[sl].reshape(4, 128, S * NT).astype(bf16),
        })

    results = _run_device(inmaps)

    num = 0.0
    den = float(lens.sum())
    # inverse slot map
    rr_all = np.arange(RPC)
    col_a, g_a = rr_all // G, rr_all % G
    for k in range(NC):
        sl = slice(k * 512, (k + 1) * 512)
        orow = np.asarray(results[k]["orow"], np.float64)   # [CH, G, F]
        oreal = np.asarray(results[k]["oreal"], np.float64).reshape(512)
        tot = np.zeros(512, np.float64)
        for c in range(CH):
            tot[c * RPC + rr_all] = orow[c, g_a, col_a]
        lc = lens[sl].astype(np.float64)
        num += float((tot + lc * kappa - oreal - hostr[sl]).sum())
    return np.float32(num / den)


def _crf_parts_np(bert_encode, transitions, output_mask, tags):
    """Numpy fallback: returns full loss."""
    ntag = NT
    start, end = ntag, ntag + 1
    maskf = output_mask.astype(np.float32)
    lengths = output_mask.sum(-1).astype(np.int64)
    b = bert_encode.shape[0]
    ar = np.arange(b)
    emit = np.take_along_axis(
        bert_encode, tags[..., None].astype(np.int64), axis=-1)[..., 0]
    emit_score = (emit * maskf).sum(-1)
    first_trans = transitions[start, tags[:, 0]]
    mid = transitions[tags[:, :-1], tags[:, 1:]]
    mid_score = (mid * maskf[:, 1:]).sum(-1)
    last_tag = tags[ar, lengths - 1]
    last_trans = transitions[last_tag, end]
    real = emit_score + first_trans + mid_score + last_trans

    trans_tt = transitions[:ntag, :ntag]
    alpha = transitions[start, :ntag][None, :] + bert_encode[:, 0, :]
    for t in range(1, bert_encode.shape[1]):
        em = bert_encode[:, t, :]
        x = alpha[:, :, None] + trans_tt[None, :, :] + em[:, None, :]
        m = x.max(axis=1)
        new = m + np.log(np.exp(x - m[:, None, :]).sum(axis=1))
        upd = output_mask[:, t] > 0
        alpha = np.where(upd[:, None], new, alpha)
    x = alpha + transitions[:ntag, end][None, :]
    m = x.max(axis=-1)
    total = m + np.log(np.exp(x - m[:, None]).sum(-1))
    return np.float32((total - real).sum() / maskf.sum())


def kernel(bert_encode, output_mask, tags, transitions):
    be = np.asarray(bert_encode, dtype=np.float32)
    om = np.asarray(output_mask, dtype=np.int32)
    tg = np.asarray(tags).astype(np.int64)
    T = np.asarray(transitions, dtype=np.float32)
    try:
        return _kernel_bass(be, om, tg.astype(np.int32), T)
    except Exception:
        import traceback
        traceback.print_exc()
        return _crf_parts_np(be, T, om, tg)
